# revision 59
# baseline (speedup 1.0000x reference)
"""Multi-head attention Bass/Tile kernel for Trainium2 (8 NeuronCores).

Problem: B=32, NQ=NK=512, IN_DIM=512, H=8 heads, E=64, OUT_DIM=512, fp32.
Sharding: data-parallel over batch — 8 cores x 4 batches, all 8 heads local
per core, so no collectives are needed.

Matmuls run in bfloat16 by default (1 PE cycle/row like float32r, but half
the SBUF/DMA/LDWEIGHTS traffic and lower PE power -> less HW throttling).
Set MHA_DTYPE=f32r for the higher-precision float32r variant. Final L2 rel
err: ~4e-3 (bf16) vs ~3e-4 (f32r); gate is 2e-2.

Dataflow per (core, batch):
  host supplies qT/kT/vT in [d, tokens] layout; b=0 input chunks are fused
  with the weight chunks (wqf/wkf) so the first matmul waits on one DMA.
  QT[he, q]  = wq[d, he].T @ qT[d, q]        (4 d-chunks, head-pair groups)
  KT[he, k]  = wk.T @ kT
  V[k, he]   = vT[d, k].T @ wv[d, he]; masked k rows scaled to 0, written
               into Vaug[k, h*128 + {pad|e}]: each head's 128-wide lhsT is
               [64 cols of (1-mask) | 64 cols of V], so the AV matmul emits
               the softmax denominator in PSUM partitions 0:64 for free
               (64-wide pad because PSUM partition bases must be 0/64).
  scoresT[k, q] = KT_h[:, ktile].T @ QT_h    (per head, 4 k-tiles; emitted
               2 heads ahead of AV so the PE never waits on exp/normalize)
  exT = exp(0.125 * scoresT)                 (ACT; mask handled via V rows)
  up[den|U, q] = Vaug_h[k, 128].T @ exT[k, q] (accumulate 4 k-chunks)
  rcp[64, q] = reciprocal_approx_fast(up[0:64])  (the 64 pad partitions are
               bitwise-identical denominator copies -> no broadcast needed)
  Unorm      = up[64:128] * rcp
  out tile   = Unorm[:, chunk, qtile].T @ wo[chunk, :]  (accumulate 4
               chunks; for the last batch chunks 0..2 are emitted between
               AV h6 and AV h7 to shorten the serial tail)

Scheduling notes (all measured on HW):
  - Sustained PE activity trips a 50%-duty power throttle; bf16 (vs f32r)
    largely avoids it. Bulk work on GpSimd downclocks the whole chip —
    keep GpSimd idle.
  - A PE warmup matmul chain before the first DMA-gated matmul lets the
    DVFS p-state ramp finish early.
"""

import os
import sys
import types

sys.path.insert(0, "/opt/trn_rl_repo")

import numpy as np

B, NQ, NK, DIN, H, E, DOUT = 32, 512, 512, 512, 8, 64, 512
NCORES = 8
BPC = B // NCORES  # batches per core
P = 128
C = DIN // P        # contraction chunks
T = NK // P         # k tiles
G = H // 2          # head-pair groups
EPAD = 64           # ones/denominator pad section (PSUM base must be 0 or 64)
E1 = E + EPAD       # V columns per head incl. padded denominator section

_CACHE = {}
LAST_RESULT = None


def _round_f32r(a):
    """Round fp32 to float32r (RNE to 11 mantissa bits, low 12 bits zero)."""
    u = np.ascontiguousarray(a, np.float32).view(np.uint32)
    r = (u + np.uint32(0x7FF) + ((u >> np.uint32(12)) & np.uint32(1))) & np.uint32(
        0xFFFFF000
    )
    return r.view(np.float32)


def _install_ntff_hook():
    """trn_boot can't register the NTFF profile hook (antenv stub lacks
    axon_hooks); recreate the module so BASS_TRACE=1 profiling works."""
    if "antenv.axon_hooks" in sys.modules:
        return
    try:
        import antenv

        mod = types.ModuleType("antenv.axon_hooks")
        holder = [None]
        mod.set_axon_ntff_profile_hook = lambda h: holder.__setitem__(0, h)
        mod.get_axon_ntff_profile_hook = lambda: holder[0]
        sys.modules["antenv.axon_hooks"] = mod
        antenv.axon_hooks = mod
        if "/root/.axon_site" not in sys.path:
            sys.path.append("/root/.axon_site")
        from trn_agent_boot.trn_boot import _ntff_profile_via_ctypes

        mod.set_axon_ntff_profile_hook(
            _ntff_profile_via_ctypes("/opt/axon/libaxon_pjrt.so")
        )
    except Exception:
        pass


def _build(mode="bf16", use_fp8=False):
    import concourse.bass as bass  # noqa: F401
    import concourse.mybir as mybir
    import concourse.tile as tile
    from concourse import bacc

    f32 = mybir.dt.float32
    f8 = mybir.dt.float8e4
    mdt = {"bf16": mybir.dt.bfloat16, "f32r": mybir.dt.float32r,
           "f32": mybir.dt.float32}[mode]
    odt = mybir.dt.bfloat16 if mode == "bf16" else f32
    sdt = f8 if use_fp8 else mdt  # score-path (Q/K) dtype

    nc = bacc.Bacc("TRN2", target_bir_lowering=False, debug=False,
                   num_devices=NCORES)

    qT = nc.dram_tensor("qT", [BPC, DIN, NQ], mdt, kind="ExternalInput")
    kT = nc.dram_tensor("kT", [BPC, DIN, NK], mdt, kind="ExternalInput")
    # vTm carries each batch's V operand with the (1-mask) row appended
    # ([P, C*NK | T]) so one DMA (and one semaphore) feeds both.
    vTm = nc.dram_tensor("vTm", [BPC, P, C * NK + T], mdt,
                         kind="ExternalInput")
    # wqf/wkf fuse each weight chunk with the matching batch-0 input chunk
    # ([P, H*E | NQ]) so the first projection matmul is gated by a single
    # DMA transfer instead of two serially-issued ones.
    wqf = nc.dram_tensor("wqf", [C, P, H * E + NQ], mdt, kind="ExternalInput")
    wkf = nc.dram_tensor("wkf", [C, P, H * E + NK], mdt, kind="ExternalInput")
    wv = nc.dram_tensor("wv", [DIN, H * E], mdt, kind="ExternalInput")
    wo = nc.dram_tensor("wo", [H * E, DOUT], mdt, kind="ExternalInput")
    out = nc.dram_tensor("out", [BPC, NQ, DOUT], odt, kind="ExternalOutput")

    with tile.TileContext(nc) as tc:
        with (
            tc.tile_pool(name="consts", bufs=1) as cpool,
            tc.tile_pool(name="io", bufs=3) as iopool,
            tc.tile_pool(name="work", bufs=2) as wpool,
            tc.tile_pool(name="expool", bufs=5) as expool,
            tc.tile_pool(name="ps_big", bufs=3, space="PSUM") as ps_big,
            tc.tile_pool(name="ps_small", bufs=2, space="PSUM") as ps_small,
        ):
            # ---- constants (weight chunk tiles; b0 DMAs interleaved with
            # input chunks so the first matmul starts after ~1.5MB, not 7MB)
            wq_sbs = [cpool.tile([P, H * E + NQ], mdt, name=f"wq{c}", tag=f"wq{c}") for c in range(C)]
            wk_sbs = [cpool.tile([P, H * E + NK], mdt, name=f"wk{c}", tag=f"wk{c}") for c in range(C)]
            wv_sbs = [cpool.tile([P, H * E], mdt, name=f"wv{c}", tag=f"wv{c}") for c in range(C)]
            wo_sbs = [cpool.tile([P, DOUT], mdt, name=f"wo{c}", tag=f"wo{c}") for c in range(C)]
            wv_r = wv[:].rearrange("(c p) n -> c p n", p=P)
            wo_r = wo[:].rearrange("(c p) n -> c p n", p=P)

            # PE warmup: dummy matmuls bridge the idle window while the first
            # input DMAs land, so the Tensor engine's DVFS p-state ramp
            # (max clock after ~3us of continuous execution) completes before
            # the real projections start.
            warm = cpool.tile([P, NQ], mdt, tag="warm")
            nc.vector.memset(warm[:], 0.0)
            wps = ps_small.tile([P, NQ], f32, tag="ps_s")
            for _ in range(14):
                nc.tensor.matmul(wps[:], lhsT=warm[:, 0:P], rhs=warm[:],
                                 start=True, stop=True)



            for b in range(BPC):
                if b > 0:
                    qT_sbs = [iopool.tile([P, NQ], mdt, name=f"qTc{c}", tag=f"qT{c}") for c in range(C)]
                    kT_sbs = [iopool.tile([P, NK], mdt, name=f"kTc{c}", tag=f"kT{c}") for c in range(C)]
                vtm = iopool.tile([P, C * NK + T], mdt, tag="vT")
                MNB = C * NK
                # tensor_scalar requires f32 per-partition scalars
                mn32 = iopool.tile([P, T], f32, tag="mn32")
                # b=0 startup: fused (weight|input) chunk transfers, split
                # across the Sync and ACT HWDGE queues so the first Q-proj
                # matmul is gated by a single 256KB transfer. Later batches
                # are prefetched a full batch ahead, so one DMA per tensor
                # is enough (fewer semaphores -> shorter kernel epilogue).
                if b == 0:
                    for c in range(C):
                        eng = nc.sync if c < 2 else nc.scalar
                        eng.dma_start(wq_sbs[c][:], wqf[c])
                    for c in range(C):
                        eng = nc.sync if c < 2 else nc.scalar
                        eng.dma_start(wk_sbs[c][:], wkf[c])
                else:
                    qT_r = qT[b].rearrange("(c p) n -> c p n", p=P)
                    kT_r = kT[b].rearrange("(c p) n -> c p n", p=P)
                    for c in range(C):
                        nc.sync.dma_start(qT_sbs[c][:], qT_r[c])
                    for c in range(C):
                        nc.sync.dma_start(kT_sbs[c][:], kT_r[c])
                if b == 0:
                    for c in range(C):
                        nc.scalar.dma_start(wv_sbs[c][:], wv_r[c])
                nc.sync.dma_start(vtm[:], vTm[b])
                nc.vector.tensor_copy(out=mn32[:], in_=vtm[:, MNB:MNB + T])
                if b == 0:
                    for c in range(C):
                        nc.scalar.dma_start(wo_sbs[c][:], wo_r[c])

                QT_sb = wpool.tile([P, G, NQ], sdt, tag="QT")
                KT_sb = wpool.tile([P, G, NK], sdt, tag="KT")
                if use_fp8:
                    # e-pair-interleaved fp8 copies for DoubleRow score
                    # matmuls: partition p holds rows e=2p / e=2p+1 in its
                    # two free halves (repacked by an SBUF->SBUF DMA).
                    QTdr = wpool.tile([EPAD, G, 2, NQ], f8, tag="QTd")
                    KTdr = wpool.tile([EPAD, G, 2, NK], f8, tag="KTd")

                Vaug = wpool.tile([P, T, H * E1], mdt, tag="Va")
                Unorm = wpool.tile([P, G, NQ], mdt, tag="Un")

                # ---- Q/K projections: per head-pair group g -> [128(2h,e), NQ]
                for half in range(2):
                    pq = ps_big.tile([P, 2 * NQ], f32, tag="ps")
                    pk = ps_big.tile([P, 2 * NK], f32, tag="ps")
                    for gg in range(2):
                        g = 2 * half + gg
                        gs = slice(g * P, (g + 1) * P)
                        for c in range(C):
                            rhs_q = (wq_sbs[c][:, H * E:] if b == 0
                                     else qT_sbs[c][:])
                            nc.tensor.matmul(
                                pq[:, gg * NQ:(gg + 1) * NQ],
                                lhsT=wq_sbs[c][:, gs], rhs=rhs_q,
                                start=(c == 0), stop=(c == C - 1))
                        for c in range(C):
                            rhs_k = (wk_sbs[c][:, H * E:] if b == 0
                                     else kT_sbs[c][:])
                            nc.tensor.matmul(
                                pk[:, gg * NK:(gg + 1) * NK],
                                lhsT=wk_sbs[c][:, gs], rhs=rhs_k,
                                start=(c == 0), stop=(c == C - 1))
                    nc.vector.tensor_copy(
                        out=QT_sb[:, 2 * half:2 * half + 2, :],
                        in_=pq[:].rearrange("p (g n) -> p g n", g=2))
                    nc.vector.tensor_copy(
                        out=KT_sb[:, 2 * half:2 * half + 2, :],
                        in_=pk[:].rearrange("p (g n) -> p g n", g=2))
                    if use_fp8:
                        for gg in range(2):
                            g = 2 * half + gg
                            nc.sync.dma_start(QTdr[:, g], QT_sb[:, g, :])
                            nc.sync.dma_start(KTdr[:, g], KT_sb[:, g, :])

                def emit_scores_exp(h):
                    g, hh = h // 2, h % 2
                    es = slice(hh * E, (hh + 1) * E)
                    es32 = slice(hh * (E // 2), (hh + 1) * (E // 2))
                    exT = expool.tile([P, T, NQ], mdt, name="exT", tag="ex")

                    def score_mm(dst, t):
                        if use_fp8:
                            nc.tensor.matmul(
                                dst,
                                lhsT=KTdr[es32, g, :, t * P:(t + 1) * P],
                                rhs=QTdr[es32, g], start=True, stop=True,
                                perf_mode=mybir.MatmulPerfMode.DoubleRow)
                        else:
                            nc.tensor.matmul(
                                dst,
                                lhsT=KT_sb[es, g, t * P:(t + 1) * P],
                                rhs=QT_sb[es, g, :], start=True, stop=True)

                    sc0 = ps_big.tile([P, 2 * NQ], f32, name="sc0", tag="ps")
                    for t in range(2):
                        score_mm(sc0[:, t * NQ:(t + 1) * NQ], t)
                    nc.scalar.activation(
                        exT[:, 0:2, :], sc0[:].rearrange("p (t n) -> p t n", t=2),
                        mybir.ActivationFunctionType.Exp, scale=0.125)
                    sc1 = ps_big.tile([P, 2 * NQ], f32, name="sc1", tag="ps")
                    for t in range(2, T):
                        score_mm(sc1[:, (t - 2) * NQ:(t - 1) * NQ], t)
                    nc.scalar.activation(
                        exT[:, 2:4, :], sc1[:].rearrange("p (t n) -> p t n", t=2),
                        mybir.ActivationFunctionType.Exp, scale=0.125)
                    return exT

                # ---- head-0/1/2 scores first: interleaving the score/exp
                # work between the projection bursts avoids a long continuous
                # PE burst that trips the HW power throttle, and gives the PE
                # score work to chew on while the DVE finishes the Vaug
                # mask-apply after the V projection.
                # Vaug pad sections ((1-mask) -> softmax denominator) depend
                # only on the mask DMA; they sit after the Q/K casts in the
                # DVE queue (so next-batch scores aren't delayed) but well
                # before the pv -> AV critical window.
                for t in range(T):
                    va_t = Vaug[:, t, :].rearrange("p (h e) -> p h e", e=E1)
                    nc.vector.tensor_copy(
                        out=va_t[:, :, 0:EPAD],
                        in_=vtm[:, MNB + t:MNB + t + 1, None].to_broadcast(
                            (P, H, EPAD)))

                exTs = [emit_scores_exp(0), emit_scores_exp(1)]

                # ---- V projection -> Vaug with masked rows zeroed + ones
                # col; a score-prefetch between the halves keeps the PE fed
                # while the DVE applies the mask to each finished half.
                for half in range(2):
                    pv = ps_big.tile([P, 2 * H * E], f32, tag="ps")
                    for tt in range(2):
                        t = 2 * half + tt
                        for c in range(C):
                            nc.tensor.matmul(
                                pv[:, tt * H * E:(tt + 1) * H * E],
                                lhsT=vtm[:, c * NK + t * P:c * NK + (t + 1) * P],
                                rhs=wv_sbs[c][:],
                                start=(c == 0), stop=(c == C - 1))
                    for tt in range(2):
                        t = 2 * half + tt
                        va_t = Vaug[:, t, :].rearrange("p (h e) -> p h e", e=E1)
                        nc.vector.tensor_scalar_mul(
                            va_t[:, :, EPAD:E1],
                            pv[:, tt * H * E:(tt + 1) * H * E].rearrange(
                                "p (h e) -> p h e", e=E),
                            mn32[:, t:t + 1])
                # ---- per-head attention (scores run 2 heads ahead of AV so
                # the PE never waits on the exp/normalize chain) ----
                for h in range(H):
                    g, hh = h // 2, h % 2
                    es = slice(hh * E, (hh + 1) * E)
                    exT = exTs[h]
                    if h + 2 < H:
                        exTs.append(emit_scores_exp(h + 2))

                    up = ps_small.tile([P, NQ], f32, tag="ps_s")
                    for t in range(T):
                        nc.tensor.matmul(
                            up[0:E1, :],
                            lhsT=Vaug[:, t, h * E1:(h + 1) * E1],
                            rhs=exT[:, t, :],
                            start=(t == 0), stop=(t == T - 1))

                    # up[0:EPAD] holds EPAD bitwise-identical copies of the
                    # softmax denominator (every Vaug pad column is 1-mask),
                    # so the reciprocal can run 64-partitions-wide directly —
                    # no partition broadcast needed.
                    from concourse.dve_ops import (
                        RECIP_APPROX_FAST_CONSTS as _rc,
                        RECIPROCAL_APPROX_FAST as _rf,
                    )
                    rcp = wpool.tile([E, NQ], mdt, tag="rcp")
                    nc.vector._custom_dve(_rf, out=rcp[:], in0=up[0:EPAD, :],
                                          s0=_rc["s0"], s1=_rc["s1"],
                                          imm2=_rc["imm2"])
                    nc.vector.tensor_mul(out=Unorm[es, g, :],
                                         in0=up[EPAD:E1, :], in1=rcp[:])

                    # Last batch: nothing overlaps the final out-projection,
                    # so emit its head-pair chunks 0..2 between AV h6 and
                    # AV h7 (their Unorm slices are ready by then); only the
                    # chunk-3 accumulation remains after the head-7 chain.
                    if b == BPC - 1 and h == H - 2:
                        po2s = [ps_big.tile([P, 2 * DOUT], f32, tag="ps",
                                            name=f"po2{i}") for i in range(2)]
                        for c in range(C - 1):
                            for pair in range(2):
                                for j in range(2):
                                    qt = 2 * pair + j
                                    nc.tensor.matmul(
                                        po2s[pair][:, j * DOUT:(j + 1) * DOUT],
                                        lhsT=Unorm[:, c, qt * P:(qt + 1) * P],
                                        rhs=wo_sbs[c][:],
                                        start=(c == 0), stop=False,
                                        skip_group_check=True)

                # ---- output projection (bf16 staging halves the out DMA) ----
                if b == BPC - 1:
                    for pair in range(2):
                        for j in range(2):
                            qt = 2 * pair + j
                            nc.tensor.matmul(
                                po2s[pair][:, j * DOUT:(j + 1) * DOUT],
                                lhsT=Unorm[:, C - 1, qt * P:(qt + 1) * P],
                                rhs=wo_sbs[C - 1][:],
                                start=False, stop=True,
                                skip_group_check=True)
                    for pair in range(2):
                        ob2 = iopool.tile([P, 2, DOUT], odt, tag=f"ob2{pair}")
                        # pair 0 on ACT, pair 1 on DVE: the two final copies
                        # run in parallel instead of serializing the tail.
                        eng = nc.scalar if pair == 0 else nc.vector
                        if pair == 0:
                            eng.copy(
                                out=ob2[:],
                                in_=po2s[pair][:].rearrange(
                                    "p (i n) -> p i n", i=2))
                        else:
                            eng.tensor_copy(
                                out=ob2[:],
                                in_=po2s[pair][:].rearrange(
                                    "p (i n) -> p i n", i=2))
                        nc.sync.dma_start(
                            out[b, pair * 2 * P:(pair + 1) * 2 * P, :]
                            .rearrange("(i p) n -> p i n", p=P),
                            ob2[:])
                else:
                    for qt in range(NQ // P):
                        po = ps_small.tile([P, DOUT], f32, tag="ps_s")
                        for c in range(C):
                            nc.tensor.matmul(
                                po[:], lhsT=Unorm[:, c, qt * P:(qt + 1) * P],
                                rhs=wo_sbs[c][:],
                                start=(c == 0), stop=(c == C - 1))
                        ob = iopool.tile([P, DOUT], odt, tag="ob")
                        nc.scalar.copy(out=ob[:], in_=po[:])
                        nc.sync.dma_start(out[b, qt * P:(qt + 1) * P, :], ob[:])

    nc.compile()
    return nc


def kernel(q, k, v, mask, W_query, W_key, W_val, W_out):
    global LAST_RESULT
    _install_ntff_hook()
    from concourse.bass_utils import run_bass_kernel_spmd

    mode = os.environ.get("MHA_DTYPE", "bf16")
    # fp8 DoubleRow scores measured ~20% slower on HW than bf16 (the modeled
    # 0.5 cycles/row does not materialize for this shape) — off by default.
    use_fp8 = mode == "bf16" and os.environ.get("MHA_FP8", "0") == "1"
    key = ("nc", mode, use_fp8)
    if key not in _CACHE:
        _CACHE[key] = _build(mode, use_fp8)
    nc = _CACHE[key]

    if mode == "bf16":
        import ml_dtypes

        rnd = lambda a: np.asarray(a, np.float32).astype(ml_dtypes.bfloat16)
    elif mode == "f32r":
        rnd = _round_f32r
    else:
        rnd = lambda a: np.ascontiguousarray(a, np.float32)

    q = np.asarray(q, np.float32)
    k = np.asarray(k, np.float32)
    v = np.asarray(v, np.float32)
    wq_h = rnd(np.asarray(W_query, np.float32).transpose(1, 0, 2).reshape(DIN, H * E))
    wk_h = rnd(np.asarray(W_key, np.float32).transpose(1, 0, 2).reshape(DIN, H * E))
    wv_h = rnd(np.asarray(W_val, np.float32).transpose(1, 0, 2).reshape(DIN, H * E))
    wo_h = rnd(np.asarray(W_out, np.float32).reshape(H * E, DOUT))
    mn_full = (~np.asarray(mask, bool)).astype(np.float32)  # [B, NK]

    wq_c = np.asarray(wq_h).reshape(C, P, H * E)
    wk_c = np.asarray(wk_h).reshape(C, P, H * E)
    in_maps = []
    for i in range(NCORES):
        sl = slice(i * BPC, (i + 1) * BPC)
        qT_i = rnd(q[sl].transpose(0, 2, 1))
        kT_i = rnd(k[sl].transpose(0, 2, 1))
        vT_i = rnd(v[sl].transpose(0, 2, 1)).reshape(BPC, C, P, NK)
        vT_i = vT_i.transpose(0, 2, 1, 3).reshape(BPC, P, C * NK)
        mn_i = rnd(mn_full[sl].reshape(BPC, T, P).transpose(0, 2, 1))
        in_maps.append({
            "qT": qT_i,
            "kT": kT_i,
            "vTm": np.ascontiguousarray(np.concatenate([vT_i, mn_i], axis=2)),
            "wqf": np.ascontiguousarray(np.concatenate(
                [wq_c, qT_i[0].reshape(C, P, NQ)], axis=2)),
            "wkf": np.ascontiguousarray(np.concatenate(
                [wk_c, kT_i[0].reshape(C, P, NK)], axis=2)),
            "wv": wv_h, "wo": wo_h,
        })

    res = run_bass_kernel_spmd(nc, in_maps, core_ids=list(range(NCORES)))
    LAST_RESULT = res
    return np.concatenate(
        [np.asarray(r["out"], np.float32) for r in res.results], axis=0)



# revision 60
# speedup vs baseline: 1.0436x; 1.0436x over previous
"""Multi-head attention Bass/Tile kernel for Trainium2 (8 NeuronCores).

Problem: B=32, NQ=NK=512, IN_DIM=512, H=8 heads, E=64, OUT_DIM=512, fp32.
Sharding: data-parallel over batch — 8 cores x 4 batches, all 8 heads local
per core, so no collectives are needed.

Matmuls run in bfloat16 by default (1 PE cycle/row like float32r, but half
the SBUF/DMA/LDWEIGHTS traffic and lower PE power -> less HW throttling).
Set MHA_DTYPE=f32r for the higher-precision float32r variant. Final L2 rel
err: ~4e-3 (bf16) vs ~3e-4 (f32r); gate is 2e-2.

Dataflow per (core, batch):
  host supplies qT/kT/vT in [d, tokens] layout; b=0 input chunks are fused
  with the weight chunks (wqf/wkf) so the first matmul waits on one DMA.
  QT[he, q]  = wq[d, he].T @ qT[d, q]        (4 d-chunks, head-pair groups)
  KT[he, k]  = wk.T @ kT
  V[k, he]   = vT[d, k].T @ wv[d, he]; masked k rows scaled to 0, written
               into Vaug[k, h*128 + {pad|e}]: each head's 128-wide lhsT is
               [64 cols of (1-mask) | 64 cols of V], so the AV matmul emits
               the softmax denominator in PSUM partitions 0:64 for free
               (64-wide pad because PSUM partition bases must be 0/64).
  scoresT[k, q] = KT_h[:, ktile].T @ QT_h    (per head, 4 k-tiles; emitted
               2 heads ahead of AV so the PE never waits on exp/normalize)
  exT = exp(0.125 * scoresT)                 (ACT; mask handled via V rows)
  up[den|U, q] = Vaug_h[k, 128].T @ exT[k, q] (accumulate 4 k-chunks)
  rcp[64, q] = reciprocal_approx_fast(up[0:64])  (the 64 pad partitions are
               bitwise-identical denominator copies -> no broadcast needed)
  Unorm      = up[64:128] * rcp
  out tile   = Unorm[:, chunk, qtile].T @ wo[chunk, :]  (accumulate 4
               chunks; for the last batch chunks 0..2 are emitted between
               AV h6 and AV h7 to shorten the serial tail)

Scheduling notes (all measured on HW):
  - Sustained PE activity trips a 50%-duty power throttle; bf16 (vs f32r)
    largely avoids it. Bulk work on GpSimd downclocks the whole chip —
    keep GpSimd idle.
  - A PE warmup matmul chain before the first DMA-gated matmul lets the
    DVFS p-state ramp finish early.
"""

import os
import sys
import types

sys.path.insert(0, "/opt/trn_rl_repo")

import numpy as np

B, NQ, NK, DIN, H, E, DOUT = 32, 512, 512, 512, 8, 64, 512
NCORES = 8
BPC = B // NCORES  # batches per core
P = 128
C = DIN // P        # contraction chunks
T = NK // P         # k tiles
G = H // 2          # head-pair groups
EPAD = 64           # ones/denominator pad section (PSUM base must be 0 or 64)
E1 = E + EPAD       # V columns per head incl. padded denominator section

_CACHE = {}
LAST_RESULT = None


def _round_f32r(a):
    """Round fp32 to float32r (RNE to 11 mantissa bits, low 12 bits zero)."""
    u = np.ascontiguousarray(a, np.float32).view(np.uint32)
    r = (u + np.uint32(0x7FF) + ((u >> np.uint32(12)) & np.uint32(1))) & np.uint32(
        0xFFFFF000
    )
    return r.view(np.float32)


def _install_ntff_hook():
    """trn_boot can't register the NTFF profile hook (antenv stub lacks
    axon_hooks); recreate the module so BASS_TRACE=1 profiling works."""
    if "antenv.axon_hooks" in sys.modules:
        return
    try:
        import antenv

        mod = types.ModuleType("antenv.axon_hooks")
        holder = [None]
        mod.set_axon_ntff_profile_hook = lambda h: holder.__setitem__(0, h)
        mod.get_axon_ntff_profile_hook = lambda: holder[0]
        sys.modules["antenv.axon_hooks"] = mod
        antenv.axon_hooks = mod
        if "/root/.axon_site" not in sys.path:
            sys.path.append("/root/.axon_site")
        from trn_agent_boot.trn_boot import _ntff_profile_via_ctypes

        mod.set_axon_ntff_profile_hook(
            _ntff_profile_via_ctypes("/opt/axon/libaxon_pjrt.so")
        )
    except Exception:
        pass


def _build(mode="bf16", use_fp8=False):
    import concourse.bass as bass  # noqa: F401
    import concourse.mybir as mybir
    import concourse.tile as tile
    from concourse import bacc

    f32 = mybir.dt.float32
    f8 = mybir.dt.float8e4
    mdt = {"bf16": mybir.dt.bfloat16, "f32r": mybir.dt.float32r,
           "f32": mybir.dt.float32}[mode]
    odt = mybir.dt.bfloat16 if mode == "bf16" else f32
    sdt = f8 if use_fp8 else mdt  # score-path (Q/K) dtype

    nc = bacc.Bacc("TRN2", target_bir_lowering=False, debug=False,
                   num_devices=NCORES)

    qT = nc.dram_tensor("qT", [BPC, DIN, NQ], mdt, kind="ExternalInput")
    kT = nc.dram_tensor("kT", [BPC, DIN, NK], mdt, kind="ExternalInput")
    vT = nc.dram_tensor("vT", [BPC, DIN, NK], mdt, kind="ExternalInput")
    # wqf/wkf fuse each weight chunk with the matching batch-0 input chunk
    # ([P, H*E | NQ]) so the first projection matmul is gated by a single
    # DMA transfer instead of two serially-issued ones.
    wqf = nc.dram_tensor("wqf", [C, P, H * E + NQ], mdt, kind="ExternalInput")
    wkf = nc.dram_tensor("wkf", [C, P, H * E + NK], mdt, kind="ExternalInput")
    wv = nc.dram_tensor("wv", [DIN, H * E], mdt, kind="ExternalInput")
    wo = nc.dram_tensor("wo", [H * E, DOUT], mdt, kind="ExternalInput")
    mnot = nc.dram_tensor("mnot", [BPC, P, T], f32, kind="ExternalInput")
    out = nc.dram_tensor("out", [BPC, NQ, DOUT], odt, kind="ExternalOutput")

    with tile.TileContext(nc) as tc:
        with (
            tc.tile_pool(name="consts", bufs=1) as cpool,
            tc.tile_pool(name="io", bufs=3) as iopool,
            tc.tile_pool(name="work", bufs=2) as wpool,
            tc.tile_pool(name="expool", bufs=5) as expool,
            tc.tile_pool(name="ps_big", bufs=3, space="PSUM") as ps_big,
            tc.tile_pool(name="ps_small", bufs=2, space="PSUM") as ps_small,
        ):
            # ---- constants (weight chunk tiles; b0 DMAs interleaved with
            # input chunks so the first matmul starts after ~1.5MB, not 7MB)
            wq_sbs = [cpool.tile([P, H * E + NQ], mdt, name=f"wq{c}", tag=f"wq{c}") for c in range(C)]
            wk_sbs = [cpool.tile([P, H * E + NK], mdt, name=f"wk{c}", tag=f"wk{c}") for c in range(C)]
            wv_sbs = [cpool.tile([P, H * E], mdt, name=f"wv{c}", tag=f"wv{c}") for c in range(C)]
            wo_sbs = [cpool.tile([P, DOUT], mdt, name=f"wo{c}", tag=f"wo{c}") for c in range(C)]
            wv_r = wv[:].rearrange("(c p) n -> c p n", p=P)
            wo_r = wo[:].rearrange("(c p) n -> c p n", p=P)

            # PE warmup: dummy matmuls bridge the idle window while the first
            # input DMAs land, so the Tensor engine's DVFS p-state ramp
            # (max clock after ~3us of continuous execution) completes before
            # the real projections start.
            warm = cpool.tile([P, NQ], mdt, tag="warm")
            nc.vector.memset(warm[:], 0.0)
            wps = ps_small.tile([P, NQ], f32, tag="ps_s")
            for _ in range(14):
                nc.tensor.matmul(wps[:], lhsT=warm[:, 0:P], rhs=warm[:],
                                 start=True, stop=True)



            for b in range(BPC):
                if b > 0:
                    qT_sbs = [iopool.tile([P, NQ], mdt, name=f"qTc{c}", tag=f"qT{c}") for c in range(C)]
                    kT_sbs = [iopool.tile([P, NK], mdt, name=f"kTc{c}", tag=f"kT{c}") for c in range(C)]
                vT_sb = iopool.tile([P, C, NK], mdt, tag="vT")
                mn_sb = iopool.tile([P, T], f32, tag="mn")
                # b=0 startup: fused (weight|input) chunk transfers, split
                # across the Sync and ACT HWDGE queues so the first Q-proj
                # matmul is gated by a single 256KB transfer. Later batches
                # are prefetched a full batch ahead, so one DMA per tensor
                # is enough (fewer semaphores -> shorter kernel epilogue).
                if b == 0:
                    for c in range(C):
                        eng = nc.sync if c < 2 else nc.scalar
                        eng.dma_start(wq_sbs[c][:], wqf[c])
                    for c in range(C):
                        eng = nc.sync if c < 2 else nc.scalar
                        eng.dma_start(wk_sbs[c][:], wkf[c])
                else:
                    qT_r = qT[b].rearrange("(c p) n -> c p n", p=P)
                    kT_r = kT[b].rearrange("(c p) n -> c p n", p=P)
                    for c in range(C):
                        nc.sync.dma_start(qT_sbs[c][:], qT_r[c])
                    for c in range(C):
                        nc.sync.dma_start(kT_sbs[c][:], kT_r[c])
                if b == 0:
                    for c in range(C):
                        nc.scalar.dma_start(wv_sbs[c][:], wv_r[c])
                nc.sync.dma_start(mn_sb[:], mnot[b])
                nc.sync.dma_start(vT_sb[:], vT[b].rearrange("(c p) n -> p c n", p=P))
                if b == 0:
                    for c in range(C):
                        nc.scalar.dma_start(wo_sbs[c][:], wo_r[c])

                QT_sb = wpool.tile([P, G, NQ], sdt, tag="QT")
                KT_sb = wpool.tile([P, G, NK], sdt, tag="KT")
                if use_fp8:
                    # e-pair-interleaved fp8 copies for DoubleRow score
                    # matmuls: partition p holds rows e=2p / e=2p+1 in its
                    # two free halves (repacked by an SBUF->SBUF DMA).
                    QTdr = wpool.tile([EPAD, G, 2, NQ], f8, tag="QTd")
                    KTdr = wpool.tile([EPAD, G, 2, NK], f8, tag="KTd")

                Vaug = wpool.tile([P, T, H * E1], mdt, tag="Va")
                Unorm = wpool.tile([P, G, NQ], mdt, tag="Un")

                # ---- Q/K projections: per head-pair group g -> [128(2h,e), NQ]
                for half in range(2):
                    pq = ps_big.tile([P, 2 * NQ], f32, tag="ps")
                    pk = ps_big.tile([P, 2 * NK], f32, tag="ps")
                    for gg in range(2):
                        g = 2 * half + gg
                        gs = slice(g * P, (g + 1) * P)
                        for c in range(C):
                            rhs_q = (wq_sbs[c][:, H * E:] if b == 0
                                     else qT_sbs[c][:])
                            nc.tensor.matmul(
                                pq[:, gg * NQ:(gg + 1) * NQ],
                                lhsT=wq_sbs[c][:, gs], rhs=rhs_q,
                                start=(c == 0), stop=(c == C - 1))
                        for c in range(C):
                            rhs_k = (wk_sbs[c][:, H * E:] if b == 0
                                     else kT_sbs[c][:])
                            nc.tensor.matmul(
                                pk[:, gg * NK:(gg + 1) * NK],
                                lhsT=wk_sbs[c][:, gs], rhs=rhs_k,
                                start=(c == 0), stop=(c == C - 1))
                    nc.vector.tensor_copy(
                        out=QT_sb[:, 2 * half:2 * half + 2, :],
                        in_=pq[:].rearrange("p (g n) -> p g n", g=2))
                    nc.vector.tensor_copy(
                        out=KT_sb[:, 2 * half:2 * half + 2, :],
                        in_=pk[:].rearrange("p (g n) -> p g n", g=2))
                    if use_fp8:
                        for gg in range(2):
                            g = 2 * half + gg
                            nc.sync.dma_start(QTdr[:, g], QT_sb[:, g, :])
                            nc.sync.dma_start(KTdr[:, g], KT_sb[:, g, :])

                def emit_scores_exp(h):
                    g, hh = h // 2, h % 2
                    es = slice(hh * E, (hh + 1) * E)
                    es32 = slice(hh * (E // 2), (hh + 1) * (E // 2))
                    exT = expool.tile([P, T, NQ], mdt, name="exT", tag="ex")

                    def score_mm(dst, t):
                        if use_fp8:
                            nc.tensor.matmul(
                                dst,
                                lhsT=KTdr[es32, g, :, t * P:(t + 1) * P],
                                rhs=QTdr[es32, g], start=True, stop=True,
                                perf_mode=mybir.MatmulPerfMode.DoubleRow)
                        else:
                            nc.tensor.matmul(
                                dst,
                                lhsT=KT_sb[es, g, t * P:(t + 1) * P],
                                rhs=QT_sb[es, g, :], start=True, stop=True)

                    sc0 = ps_big.tile([P, 2 * NQ], f32, name="sc0", tag="ps")
                    for t in range(2):
                        score_mm(sc0[:, t * NQ:(t + 1) * NQ], t)
                    nc.scalar.activation(
                        exT[:, 0:2, :], sc0[:].rearrange("p (t n) -> p t n", t=2),
                        mybir.ActivationFunctionType.Exp, scale=0.125)
                    sc1 = ps_big.tile([P, 2 * NQ], f32, name="sc1", tag="ps")
                    for t in range(2, T):
                        score_mm(sc1[:, (t - 2) * NQ:(t - 1) * NQ], t)
                    nc.scalar.activation(
                        exT[:, 2:4, :], sc1[:].rearrange("p (t n) -> p t n", t=2),
                        mybir.ActivationFunctionType.Exp, scale=0.125)
                    return exT

                # ---- head-0/1/2 scores first: interleaving the score/exp
                # work between the projection bursts avoids a long continuous
                # PE burst that trips the HW power throttle, and gives the PE
                # score work to chew on while the DVE finishes the Vaug
                # mask-apply after the V projection.
                # Vaug pad sections ((1-mask) -> softmax denominator) depend
                # only on the mask DMA; they sit after the Q/K casts in the
                # DVE queue (so next-batch scores aren't delayed) but well
                # before the pv -> AV critical window.
                for t in range(T):
                    va_t = Vaug[:, t, :].rearrange("p (h e) -> p h e", e=E1)
                    nc.vector.tensor_copy(
                        out=va_t[:, :, 0:EPAD],
                        in_=mn_sb[:, t:t + 1, None].to_broadcast((P, H, EPAD)))

                exTs = [emit_scores_exp(0), emit_scores_exp(1)]

                # ---- V projection -> Vaug with masked rows zeroed + ones
                # col; a score-prefetch between the halves keeps the PE fed
                # while the DVE applies the mask to each finished half.
                for half in range(2):
                    pv = ps_big.tile([P, 2 * H * E], f32, tag="ps")
                    for tt in range(2):
                        t = 2 * half + tt
                        for c in range(C):
                            nc.tensor.matmul(
                                pv[:, tt * H * E:(tt + 1) * H * E],
                                lhsT=vT_sb[:, c, t * P:(t + 1) * P],
                                rhs=wv_sbs[c][:],
                                start=(c == 0), stop=(c == C - 1))
                    for tt in range(2):
                        t = 2 * half + tt
                        va_t = Vaug[:, t, :].rearrange("p (h e) -> p h e", e=E1)
                        nc.vector.tensor_scalar_mul(
                            va_t[:, :, EPAD:E1],
                            pv[:, tt * H * E:(tt + 1) * H * E].rearrange(
                                "p (h e) -> p h e", e=E),
                            mn_sb[:, t:t + 1])
                # ---- per-head attention (scores run 2 heads ahead of AV so
                # the PE never waits on the exp/normalize chain) ----
                for h in range(H):
                    g, hh = h // 2, h % 2
                    es = slice(hh * E, (hh + 1) * E)
                    exT = exTs[h]
                    if h + 2 < H:
                        exTs.append(emit_scores_exp(h + 2))

                    up = ps_small.tile([P, NQ], f32, tag="ps_s")
                    for t in range(T):
                        nc.tensor.matmul(
                            up[0:E1, :],
                            lhsT=Vaug[:, t, h * E1:(h + 1) * E1],
                            rhs=exT[:, t, :],
                            start=(t == 0), stop=(t == T - 1))

                    # up[0:EPAD] holds EPAD bitwise-identical copies of the
                    # softmax denominator (every Vaug pad column is 1-mask),
                    # so the reciprocal can run 64-partitions-wide directly —
                    # no partition broadcast needed.
                    from concourse.dve_ops import (
                        RECIP_APPROX_FAST_CONSTS as _rc,
                        RECIPROCAL_APPROX_FAST as _rf,
                    )
                    rcp = wpool.tile([E, NQ], mdt, tag="rcp")
                    nc.vector._custom_dve(_rf, out=rcp[:], in0=up[0:EPAD, :],
                                          s0=_rc["s0"], s1=_rc["s1"],
                                          imm2=_rc["imm2"])
                    nc.vector.tensor_mul(out=Unorm[es, g, :],
                                         in0=up[EPAD:E1, :], in1=rcp[:])

                    # Last batch: nothing overlaps the final out-projection,
                    # so emit its head-pair chunks 0..2 between AV h6 and
                    # AV h7 (their Unorm slices are ready by then); only the
                    # chunk-3 accumulation remains after the head-7 chain.
                    if b == BPC - 1 and h == H - 2:
                        po2s = [ps_big.tile([P, 2 * DOUT], f32, tag="ps",
                                            name=f"po2{i}") for i in range(2)]
                        for c in range(C - 1):
                            for pair in range(2):
                                for j in range(2):
                                    qt = 2 * pair + j
                                    nc.tensor.matmul(
                                        po2s[pair][:, j * DOUT:(j + 1) * DOUT],
                                        lhsT=Unorm[:, c, qt * P:(qt + 1) * P],
                                        rhs=wo_sbs[c][:],
                                        start=(c == 0), stop=False,
                                        skip_group_check=True)

                # ---- output projection (bf16 staging halves the out DMA) ----
                if b == BPC - 1:
                    for pair in range(2):
                        for j in range(2):
                            qt = 2 * pair + j
                            nc.tensor.matmul(
                                po2s[pair][:, j * DOUT:(j + 1) * DOUT],
                                lhsT=Unorm[:, C - 1, qt * P:(qt + 1) * P],
                                rhs=wo_sbs[C - 1][:],
                                start=False, stop=True,
                                skip_group_check=True)
                    for pair in range(2):
                        ob2 = iopool.tile([P, 2, DOUT], odt, tag=f"ob2{pair}")
                        nc.scalar.copy(
                            out=ob2[:],
                            in_=po2s[pair][:].rearrange("p (i n) -> p i n", i=2))
                        nc.sync.dma_start(
                            out[b, pair * 2 * P:(pair + 1) * 2 * P, :]
                            .rearrange("(i p) n -> p i n", p=P),
                            ob2[:])
                else:
                    for qt in range(NQ // P):
                        po = ps_small.tile([P, DOUT], f32, tag="ps_s")
                        for c in range(C):
                            nc.tensor.matmul(
                                po[:], lhsT=Unorm[:, c, qt * P:(qt + 1) * P],
                                rhs=wo_sbs[c][:],
                                start=(c == 0), stop=(c == C - 1))
                        ob = iopool.tile([P, DOUT], odt, tag="ob")
                        nc.scalar.copy(out=ob[:], in_=po[:])
                        nc.sync.dma_start(out[b, qt * P:(qt + 1) * P, :], ob[:])

    nc.compile()
    return nc


def kernel(q, k, v, mask, W_query, W_key, W_val, W_out):
    global LAST_RESULT
    _install_ntff_hook()
    from concourse.bass_utils import run_bass_kernel_spmd

    mode = os.environ.get("MHA_DTYPE", "bf16")
    # fp8 DoubleRow scores measured ~20% slower on HW than bf16 (the modeled
    # 0.5 cycles/row does not materialize for this shape) — off by default.
    use_fp8 = mode == "bf16" and os.environ.get("MHA_FP8", "0") == "1"
    key = ("nc", mode, use_fp8)
    if key not in _CACHE:
        _CACHE[key] = _build(mode, use_fp8)
    nc = _CACHE[key]

    if mode == "bf16":
        import ml_dtypes

        rnd = lambda a: np.asarray(a, np.float32).astype(ml_dtypes.bfloat16)
    elif mode == "f32r":
        rnd = _round_f32r
    else:
        rnd = lambda a: np.ascontiguousarray(a, np.float32)

    q = np.asarray(q, np.float32)
    k = np.asarray(k, np.float32)
    v = np.asarray(v, np.float32)
    wq_h = rnd(np.asarray(W_query, np.float32).transpose(1, 0, 2).reshape(DIN, H * E))
    wk_h = rnd(np.asarray(W_key, np.float32).transpose(1, 0, 2).reshape(DIN, H * E))
    wv_h = rnd(np.asarray(W_val, np.float32).transpose(1, 0, 2).reshape(DIN, H * E))
    wo_h = rnd(np.asarray(W_out, np.float32).reshape(H * E, DOUT))
    mn_full = (~np.asarray(mask, bool)).astype(np.float32)  # [B, NK]

    wq_c = np.asarray(wq_h).reshape(C, P, H * E)
    wk_c = np.asarray(wk_h).reshape(C, P, H * E)
    in_maps = []
    for i in range(NCORES):
        sl = slice(i * BPC, (i + 1) * BPC)
        qT_i = rnd(q[sl].transpose(0, 2, 1))
        kT_i = rnd(k[sl].transpose(0, 2, 1))
        in_maps.append({
            "qT": qT_i,
            "kT": kT_i,
            "vT": rnd(v[sl].transpose(0, 2, 1)),
            "wqf": np.ascontiguousarray(np.concatenate(
                [wq_c, qT_i[0].reshape(C, P, NQ)], axis=2)),
            "wkf": np.ascontiguousarray(np.concatenate(
                [wk_c, kT_i[0].reshape(C, P, NK)], axis=2)),
            "wv": wv_h, "wo": wo_h,
            "mnot": np.ascontiguousarray(
                mn_full[sl].reshape(BPC, T, P).transpose(0, 2, 1)),
        })

    res = run_bass_kernel_spmd(nc, in_maps, core_ids=list(range(NCORES)))
    LAST_RESULT = res
    return np.concatenate(
        [np.asarray(r["out"], np.float32) for r in res.results], axis=0)

